# revision 1
# baseline (speedup 1.0000x reference)
"""Trainium2 Bass kernel for nn_MultiHeadAttention (B=1, S=4096, E=768, H=12, D=64).

Causal multi-head attention, sequence-parallel across 8 NeuronCores.

Strategy (single SPMD program, per-core variation is data-only):
- Query rows are split into 32 global chunks of 128 rows. Core c owns chunks
  G(g) = 8g + c for g = 0..3. Chunk g runs a fixed slot loop of 8(g+1)
  k-blocks (uniform across cores); causally-dead slots are killed by a
  per-slot exp bias of -60 (data), and the diagonal block is handled in the
  last slot with separately-projected "own" K/V tiles plus a triangular mask.
- Scores are computed transposed (S^T[k, q], k on partitions) so the exp
  output P^T feeds the attn@V matmul directly. Row sums l come from a fused
  ones-column appended to V. 1/l = exp(-ln(l)) on the scalar engine (both
  functions live in one activation table set). The 1/l broadcast across
  partitions is a K=1 matmul against a ones vector.
- All matmuls in bf16 (fp32 PSUM accumulation); x is transposed via DMA
  xbar transpose after a bf16 cast.
- Every core computes the full K/V projection locally (replicated), its own
  Q, and its own 512 output rows including the final out-projection + bias.
"""

import os
from contextlib import ExitStack

import numpy as np
import ml_dtypes

import concourse.bass as bass
import concourse.tile as tile
from concourse import bacc, bass_utils, mybir

F32 = mybir.dt.float32
F32R = mybir.dt.float32r
BF16 = mybir.dt.bfloat16

N_CORES = 8
S, E, H, D = 4096, 768, 12, 64
P = 128
NCH = 4  # chunks per core
SLOTS = [8, 16, 24, 32]  # slot count for chunk g
SLOT_BASE = [0, 8, 24, 48]  # cumulative
TOT_SLOTS = 80
EC = E // P  # 6 e-chunks of 128
NPAIR = 6  # head pairs
QOWN = NCH * P  # 512 own q rows
NEG = -60.0  # exp bias for masked slots: exp(-60 +- 4) == 0 numerically


def build_program():
    nc = bacc.Bacc("TRN2", target_bir_lowering=False, debug=False, num_devices=N_CORES)

    x = nc.dram_tensor("x", [S, E], F32, kind="ExternalInput").ap()
    xq = nc.dram_tensor("xq", [QOWN, E], F32, kind="ExternalInput").ap()
    wq = nc.dram_tensor("wq", [E, E], F32, kind="ExternalInput").ap()
    wk = nc.dram_tensor("wk", [E, E], F32, kind="ExternalInput").ap()
    wv = nc.dram_tensor("wv", [E, E], F32, kind="ExternalInput").ap()
    wo = nc.dram_tensor("wo", [D, H, E], F32, kind="ExternalInput").ap()
    bob = nc.dram_tensor("bob", [P, E], F32, kind="ExternalInput").ap()
    btab = nc.dram_tensor("btab", [P, TOT_SLOTS], F32, kind="ExternalInput").ap()
    dmask = nc.dram_tensor("dmask", [P, NPAIR * P], BF16, kind="ExternalInput").ap()
    y = nc.dram_tensor("y", [QOWN, E], F32, kind="ExternalOutput").ap()

    with tile.TileContext(nc) as tc, ExitStack() as top:
        const = top.enter_context(tc.tile_pool(name="const", bufs=1))
        big = top.enter_context(tc.tile_pool(name="big", bufs=1))
        own = top.enter_context(tc.tile_pool(name="own", bufs=1))
        sc_p = top.enter_context(tc.tile_pool(name="scp", bufs=2, space="PSUM"))
        ctx_p = top.enter_context(tc.tile_pool(name="ctxp", bufs=2, space="PSUM"))

        # ---- constants ----
        btab_sb = const.tile([P, TOT_SLOTS], F32, tag="btab")
        nc.sync.dma_start(out=btab_sb, in_=btab)
        dmask_bf = const.tile([P, NPAIR * P], BF16, tag="dmaskb")
        nc.sync.dma_start(out=dmask_bf, in_=dmask)
        bob_sb = const.tile([P, E], F32, tag="bob")
        nc.sync.dma_start(out=bob_sb, in_=bob)
        ones_f = const.tile([P, D], F32, tag="onesf")
        nc.vector.memset(ones_f, 1.0)
        ones_sb = const.tile([P, D], F32R, tag="ones")
        nc.scalar.copy(out=ones_sb, in_=ones_f)
        zb = const.tile([P, 512], BF16, tag="zb")
        nc.vector.memset(zb, 0.0)

        # ---- persistent bf16 operands ----
        kt = big.tile([P, NPAIR, S], BF16, tag="kt")  # K^T, head pairs on partitions
        vt = big.tile([P, S // P, H * (D + 1)], BF16, tag="vt")  # V + ones cols
        # own Q^T, zero-padded variant pairs: qtp[:, pc, 0, :] has head-pair
        # rows 64:128 zeroed (selects the even head), qtp[:, pc, 1, :] has rows
        # 0:64 zeroed (odd head). Scores contract over the full 128 partitions
        # (the dead half contributes 0), keeping every matmul operand at base
        # partition 0 (base-64 operands hang this HW path), and both heads of
        # a pair ride one N=256 matmul with the same stationary K tile.
        qtp = own.tile([P, NPAIR, 2, QOWN], BF16, tag="qtp")
        nc.vector.memset(qtp[D:P, :, 0, :], 0.0)
        nc.vector.memset(qtp[0:D, :, 1, :], 0.0)
        kto = own.tile([P, NPAIR, QOWN], BF16, tag="kto")  # own K^T (diagonal)
        vto = own.tile([P, NCH, H * (D + 1)], BF16, tag="vto")  # own V (diagonal)

        vt_v = vt.rearrange("p b (h c) -> p b h c", c=D + 1)
        vto_v = vto.rearrange("p b (h c) -> p b h c", c=D + 1)

        # ================= projection phase (nested pools) =================
        with ExitStack() as proj:
            wpool = proj.enter_context(tc.tile_pool(name="wpool", bufs=1))
            wstage = proj.enter_context(tc.tile_pool(name="wstage", bufs=2))
            xst_p = proj.enter_context(tc.tile_pool(name="xst", bufs=2))
            xbf_p = proj.enter_context(tc.tile_pool(name="xbf", bufs=2))
            xt_p = proj.enter_context(tc.tile_pool(name="xt", bufs=2))
            xq_pool = proj.enter_context(tc.tile_pool(name="xqp", bufs=1))

            wqb = wpool.tile([P, EC, E], BF16, tag="wqb")
            wkb = wpool.tile([P, EC, E], BF16, tag="wkb")
            wvb = wpool.tile([P, EC, E], BF16, tag="wvb")
            for w_dram, w_bf in ((wq, wqb), (wk, wkb), (wv, wvb)):
                for h3 in range(3):  # stream in thirds
                    wst = wstage.tile([P, 2, E], F32, tag="wst")
                    nc.sync.dma_start(
                        out=wst,
                        in_=w_dram.rearrange("(c p) n -> p c n", p=P)[:, 2 * h3 : 2 * h3 + 2, :],
                    )
                    nc.vector.tensor_copy(out=w_bf[:, 2 * h3 : 2 * h3 + 2, :], in_=wst)

            xqt = xq_pool.tile([P, EC, QOWN], BF16, tag="xqt")  # own x^T

            def load_transposed(src, sblk0, nblk, dst, dst_col0):
                """DMA nblk [128,768] row-blocks, cast bf16, xbar-transpose into
                dst[:, ec, dst_col0 : dst_col0 + nblk*128]."""
                xs = xst_p.tile([P, nblk, E], F32, tag="xst")
                nc.sync.dma_start(
                    out=xs,
                    in_=src.rearrange("(s p) e -> p s e", p=P)[:, sblk0 : sblk0 + nblk, :],
                )
                xb = xbf_p.tile([P, nblk, E], BF16, tag="xbf")
                nc.vector.tensor_copy(out=xb, in_=xs)
                for b in range(nblk):
                    for ec in range(EC):
                        nc.sync.dma_start_transpose(
                            out=dst[:, ec, dst_col0 + b * P : dst_col0 + (b + 1) * P],
                            in_=xb[:, b, ec * P : (ec + 1) * P],
                        )

            for gi in range(2):
                load_transposed(xq, 2 * gi, 2, xqt, gi * 256)

            def project_pairs(w_bf, dst, src_t, ncols, split=False):
                for pc in range(NPAIR):
                    ps = sc_p.tile([P, ncols], F32, tag="sc", name=f"ps_p{pc}")
                    for ec in range(EC):
                        nc.tensor.matmul(
                            out=ps,
                            lhsT=w_bf[:, ec, pc * P : (pc + 1) * P],
                            rhs=src_t[:, ec, :],
                            start=(ec == 0),
                            stop=(ec == EC - 1),
                        )
                    if split:
                        nc.vector.tensor_copy(out=dst[0:D, pc, 0, :], in_=ps[0:D, :])
                        nc.vector.tensor_copy(out=dst[D:P, pc, 1, :], in_=ps[D:P, :])
                    else:
                        nc.scalar.copy(out=dst[:, pc, :], in_=ps)

            project_pairs(wqb, qtp, xqt, QOWN, split=True)
            project_pairs(wkb, kto, xqt, QOWN)

            def project_v(dst_view, xt_tile, xt_col0):
                for half in range(2):
                    ps = sc_p.tile([P, E // 2], F32, tag="sc", name=f"ps_v{half}")
                    for ec in range(EC):
                        nc.tensor.matmul(
                            out=ps,
                            lhsT=xt_tile[:, ec, xt_col0 : xt_col0 + P],
                            rhs=wvb[:, ec, half * 384 : (half + 1) * 384],
                            start=(ec == 0),
                            stop=(ec == EC - 1),
                        )
                    nc.vector.tensor_copy(
                        out=dst_view[:, half * 6 : (half + 1) * 6, 0:D],
                        in_=ps.rearrange("p (h d) -> p h d", d=D),
                    )
                nc.vector.memset(dst_view[:, :, D : D + 1], 1.0)

            for qb in range(NCH):
                project_v(vto_v[:, qb, :, :], xqt, qb * P)

            # ---- full K^T and V from x (the replicated part) ----
            for sg in range(S // 512):
                xt_g = xt_p.tile([P, EC, 512], BF16, tag="xtg", name=f"xt_{sg}")
                for gi in range(2):
                    load_transposed(x, 4 * sg + 2 * gi, 2, xt_g, gi * 256)
                for pc in range(NPAIR):
                    ps = sc_p.tile([P, 512], F32, tag="sc", name=f"ps_k{pc}")
                    for ec in range(EC):
                        nc.tensor.matmul(
                            out=ps,
                            lhsT=wkb[:, ec, pc * P : (pc + 1) * P],
                            rhs=xt_g[:, ec, :],
                            start=(ec == 0),
                            stop=(ec == EC - 1),
                        )
                    nc.scalar.copy(out=kt[:, pc, sg * 512 : (sg + 1) * 512], in_=ps)
                for b in range(4):
                    project_v(vt_v[:, 4 * sg + b, :, :], xt_g, b * P)

        phase = os.environ.get("KERNEL_PHASE", "full")
        if phase == "proj":
            # debug: dump projections into y and stop
            dbg = own.tile([P, E], F32, tag="dbg")
            nc.scalar.copy(out=dbg[:, 0:E], in_=kt[:, 0, 0:E])
            nc.sync.dma_start(out=y[0:P, :], in_=dbg)
            dbg2 = own.tile([P, E], F32, tag="dbg2")
            nc.scalar.copy(out=dbg2, in_=vt[:, 0, 0:E])
            nc.sync.dma_start(out=y[P : 2 * P, :], in_=dbg2)
            dbg3 = own.tile([P, E], F32, tag="dbg3")
            nc.scalar.copy(out=dbg3[:, 0:QOWN], in_=qt[:, 0, :])
            nc.scalar.copy(out=dbg3[:, QOWN:E], in_=kto[:, 1, 0 : E - QOWN])
            nc.sync.dma_start(out=y[2 * P : 3 * P, :], in_=dbg3)
            dbg4 = own.tile([P, E], F32, tag="dbg4")
            nc.scalar.copy(out=dbg4, in_=vto[:, 0, 0:E])
            nc.sync.dma_start(out=y[3 * P : 4 * P, :], in_=dbg4)

        n_chunks_run = 0 if phase == "proj" else NCH
        if phase.startswith("att"):
            n_chunks_run = int(phase[3:])
        skip_epi = bool(int(os.environ.get("KERNEL_SKIP_EPI", "0")))

        # ================= attention phase =================
        with ExitStack() as att:
            wo_pool = att.enter_context(tc.tile_pool(name="wop", bufs=1))
            wob = wo_pool.tile([D, H, E], BF16, tag="wob")
            with ExitStack() as wos:
                wo_st = wos.enter_context(tc.tile_pool(name="wost", bufs=1))
                for h2 in range(2):
                    wost = wo_st.tile([D, EC, E], F32, tag="wost", name=f"wo_{h2}")
                    nc.sync.dma_start(out=wost, in_=wo[:, 6 * h2 : 6 * h2 + 6, :])
                    nc.vector.tensor_copy(out=wob[:, 6 * h2 : 6 * h2 + 6, :], in_=wost)

            pt_p = att.enter_context(tc.tile_pool(name="pt", bufs=1))
            misc = att.enter_context(tc.tile_pool(name="misc", bufs=1))

            for g in range(n_chunks_run):
                nslot = SLOTS[g]
                ctx_t = [
                    ctx_p.tile([D + 1, NPAIR * P], F32, tag="ctx", name=f"ctx_g{g}_{i}")
                    for i in range(2)
                ]

                def ctx_fence(start):
                    # bank-wide zero matmuls fencing the per-head accumulation:
                    # one start=True / stop=True group per PSUM bank, with all
                    # real ctx matmuls as flags=0 accumulates in between. The
                    # full-bank writes give WAW deps ordering them correctly.
                    for t in ctx_t:
                        for lo, n in ((0, 512), (512, 256)):
                            nc.tensor.matmul(
                                out=t[0 : D + 1, lo : lo + n],
                                lhsT=zb[0:1, 0 : D + 1],
                                rhs=zb[0:1, 0:n],
                                start=start,
                                stop=not start,
                            )

                ctx_fence(start=True)
                for s in range(nslot):
                    is_diag = s == nslot - 1
                    for hg in range(2):
                        sc = sc_p.tile(
                            [P, NPAIR * P], F32, tag="sc", name=f"sc_{g}_{s}_{hg}"
                        )
                        for pl in range(3):
                            pc = hg * 3 + pl
                            if is_diag:
                                lhsT = kto[:, pc, g * P : (g + 1) * P]
                            else:
                                lhsT = kt[:, pc, s * P : (s + 1) * P]
                            # single-shot scores; interleaved start=True groups in
                            # one bank are HW-safe (per-element data unaffected).
                            # rhs [128, 2, 128] = both zero-padded Q variants ->
                            # out [128, 256] = both heads of the pair.
                            nc.tensor.matmul(
                                out=sc[:, pl * 2 * P : (pl + 1) * 2 * P],
                                lhsT=lhsT,
                                rhs=qtp[:, pc, :, g * P : (g + 1) * P],
                                start=True,
                                stop=True,
                                skip_group_check=True,
                            )
                        pt = pt_p.tile(
                            [P, NPAIR * P], BF16, tag="pt", bufs=2, name=f"pt_{g}_{s}_{hg}"
                        )
                        sg_idx = SLOT_BASE[g] + s
                        nc.scalar.activation(
                            out=pt,
                            in_=sc,
                            func=mybir.ActivationFunctionType.Exp,
                            bias=btab_sb[:, sg_idx : sg_idx + 1],
                            scale=0.125,
                        )
                        if is_diag:
                            ptm = pt_p.tile(
                                [P, NPAIR * P], BF16, tag="ptm", name=f"ptm_{g}_{hg}"
                            )
                            nc.vector.tensor_mul(out=ptm, in0=pt, in1=dmask_bf)
                            pt = ptm
                        for hl in range(6):
                            h = hg * 6 + hl
                            vsrc = vto_v[:, g, h, :] if is_diag else vt_v[:, s, h, :]
                            nc.tensor.matmul(
                                out=ctx_t[hg][:, hl * P : (hl + 1) * P],
                                lhsT=vsrc,
                                rhs=pt[:, hl * P : (hl + 1) * P],
                                start=False,
                                stop=False,
                            )
                ctx_fence(start=False)
                if skip_epi:
                    dbg5 = misc.tile([P, E], F32, tag="dbg5", name=f"dbg5_{g}")
                    nc.scalar.copy(out=dbg5[0 : D + 1, :], in_=ctx_t[0][:, :])
                    nc.sync.dma_start(out=y[g * P : (g + 1) * P, :], in_=dbg5)
                    continue
                # ---- epilogue: 1/l, normalize, out-projection ----
                ctxn = []
                for hg in range(2):
                    # l lives on PSUM partition 64 (the V ones-column row); PE
                    # and ACT only work from base partition 0 here, so stage it
                    # to partition 0: DVE copy (partition-locked) + tiny
                    # SBUF->SBUF DMA partition move.
                    lrow = misc.tile([P, NPAIR * P], F32, tag="lrow", name=f"lr{g}{hg}")
                    nc.vector.tensor_copy(
                        out=lrow[D : D + 1, :], in_=ctx_t[hg][D : D + 1, :]
                    )
                    nc.sync.dma_start(out=lrow[0:1, :], in_=lrow[D : D + 1, :])
                    lln = misc.tile([P, NPAIR * P], F32, tag="lln", name=f"lln{g}{hg}")
                    nc.scalar.activation(
                        out=lln[0:1, :],
                        in_=lrow[0:1, :],
                        func=mybir.ActivationFunctionType.Ln,
                    )
                    linv = misc.tile([P, NPAIR * P], F32R, tag="linv", name=f"li{g}{hg}")
                    nc.scalar.activation(
                        out=linv[0:1, :],
                        in_=lln[0:1, :],
                        func=mybir.ActivationFunctionType.Exp,
                        scale=-1.0,
                    )
                    bc = sc_p.tile([D, NPAIR * P], F32, tag="sc", name=f"bc{g}{hg}")
                    for lo, n in ((0, 512), (512, 256)):  # bank-aligned pieces
                        nc.tensor.matmul(
                            out=bc[:, lo : lo + n],
                            lhsT=ones_sb[0:1, 0:D],
                            rhs=linv[0:1, lo : lo + n],
                            start=True,
                            stop=True,
                        )
                    bcs = misc.tile(
                        [D, NPAIR * P], F32, tag="bcs", bufs=2, name=f"bcs{g}{hg}"
                    )
                    nc.scalar.copy(out=bcs, in_=bc)
                    cn = misc.tile(
                        [D, NPAIR * P], BF16, tag="ctxn", bufs=2, name=f"cn{g}{hg}"
                    )
                    nc.vector.tensor_mul(out=cn, in0=ctx_t[hg][0:D, :], in1=bcs)
                    ctxn.append(cn)
                for fh in range(2):
                    op = sc_p.tile([P, 384], F32, tag="sc", name=f"op{g}{fh}")
                    for h in range(H):
                        nc.tensor.matmul(
                            out=op,
                            lhsT=ctxn[h // 6][:, (h % 6) * P : (h % 6 + 1) * P],
                            rhs=wob[:, h, fh * 384 : (fh + 1) * 384],
                            start=(h == 0),
                            stop=(h == H - 1),
                        )
                    outs = misc.tile([P, 384], F32, tag="outs", bufs=2, name=f"ou{g}{fh}")
                    nc.vector.tensor_add(
                        out=outs, in0=op, in1=bob_sb[:, fh * 384 : (fh + 1) * 384]
                    )
                    nc.sync.dma_start(
                        out=y[g * P : (g + 1) * P, fh * 384 : (fh + 1) * 384], in_=outs
                    )

    nc.compile()
    return nc


_NC_CACHE = None


def _get_program():
    global _NC_CACHE
    if _NC_CACHE is None:
        _NC_CACHE = build_program()
    return _NC_CACHE


def _host_inputs(x, Wq, Wk, Wv, Wo, bo):
    """Build per-core input maps."""
    x = np.ascontiguousarray(x.reshape(S, E), dtype=np.float32)
    wo_arr = np.ascontiguousarray(
        Wo.reshape(H, D, E).transpose(1, 0, 2), dtype=np.float32
    )
    bob = np.ascontiguousarray(np.broadcast_to(bo.astype(np.float32), (P, E)))
    # diagonal mask, replicated per head-group lane: [k, hl*128 + q] = k <= q
    tri = (np.arange(P)[:, None] <= np.arange(P)[None, :]).astype(np.float32)
    dmask = np.ascontiguousarray(np.tile(tri, (1, NPAIR)).astype(ml_dtypes.bfloat16))

    in_maps = []
    for c in range(N_CORES):
        chunks = [8 * g + c for g in range(NCH)]
        xq = np.concatenate([x[gc * P : (gc + 1) * P] for gc in chunks], axis=0)
        btab = np.zeros((P, TOT_SLOTS), dtype=np.float32)
        for g in range(NCH):
            diagk = chunks[g]
            for s in range(SLOTS[g]):
                if s == SLOTS[g] - 1 or s < diagk:
                    v = 0.0  # diagonal slot or fully-valid block
                else:
                    v = NEG  # causally dead block
                btab[:, SLOT_BASE[g] + s] = v
        in_maps.append(
            {
                "x": x,
                "xq": np.ascontiguousarray(xq),
                "wq": np.ascontiguousarray(Wq, dtype=np.float32),
                "wk": np.ascontiguousarray(Wk, dtype=np.float32),
                "wv": np.ascontiguousarray(Wv, dtype=np.float32),
                "wo": wo_arr,
                "bob": bob,
                "btab": btab,
                "dmask": dmask,
            }
        )
    return in_maps


def kernel(x, Wq, Wk, Wv, Wo, bo, mask=None, **_ignored):
    nc = _get_program()
    in_maps = _host_inputs(
        np.asarray(x), np.asarray(Wq), np.asarray(Wk), np.asarray(Wv),
        np.asarray(Wo), np.asarray(bo),
    )
    trace = bool(int(os.environ.get("BASS_KERNEL_TRACE", "0")))
    res = bass_utils.run_bass_kernel_spmd(
        nc, in_maps, core_ids=list(range(N_CORES)), trace=trace
    )
    if trace:
        kernel.last_results = res
    out = np.empty((S, E), dtype=np.float32)
    for c in range(N_CORES):
        yc = res.results[c]["y"]
        for g in range(NCH):
            gc = 8 * g + c
            out[gc * P : (gc + 1) * P] = yc[g * P : (g + 1) * P]
    return out.reshape(1, S, E)



# revision 2
# speedup vs baseline: 290.7479x; 290.7479x over previous
"""Trainium2 Bass kernel v2 for nn_MultiHeadAttention (B=1, S=4096, E=768, H=12, D=64).

Causal MHA, sequence-parallel across 8 cores (chunk g of core c = global
q-chunk 8g+c), fp8-heavy compute:

- Q/K projections + scores in fp8e4 with DoubleRow perf mode (4x bf16
  throughput; weights pre-scaled x64 on host so fp8 dynamic range is used).
- Off-diagonal attn@V in fp8-DR over slot pairs (V from fp8 projection,
  ones-column carries the softmax denominator l).
- Diagonal (and one unpaired) slot in higher precision: P bf16, V bf16 from
  a bf16 projection - kills the early-row error spike where softmax is
  peaked and ctx inherits a single v's quantization error.
- exp split across three engines per slot: ACT does heads 0-5 natively
  (fp8 out); DVE computes heads 6-11 via the Schraudolph bit trick
  (i32 = sc*A + B, bits == fp32 exp) and Pool casts those to fp8.
- Causally dead slots killed by exp bias -60 (data-driven, uniform program).
- Out-projection bf16, softmax 1/l via Ln/Exp + f32r ones-matmul broadcast.
"""

import os

import numpy as np
import ml_dtypes

import concourse.bass as bass
import concourse.tile as tile
from concourse import bacc, bass_utils, mybir

F32 = mybir.dt.float32
F32R = mybir.dt.float32r
BF16 = mybir.dt.bfloat16
FP8 = mybir.dt.float8e4
DR = mybir.MatmulPerfMode.DoubleRow
EXP = mybir.ActivationFunctionType.Exp

N_CORES = 8
S, E, H, D = 4096, 768, 12, 64
P = 128
NCH = 4
SLOTS = [8, 16, 24, 32]
SLOT_BASE = [0, 8, 24, 48]
TOT_SLOTS = 80
QOWN = NCH * P
NEG = -60.0
VST = 68  # vt8 head stride: DR needs tile step (12*VST) 16B-aligned

SC_SCALE = 0.125 / (64.0 * 64.0)  # scores psum = 4096 * (q.k); exp scale
# int16 Schraudolph: i16 = sc*A16 + B16; i16 bits are the bf16 of exp(sc*scale)
SCHR_A = (2.0 ** 7) / np.log(2.0) * SC_SCALE
SCHR_C = 366392.0 / 65536.0
SCHR_B0 = 127.0 * 2.0 ** 7 - SCHR_C
LN64 = float(np.log(64.0))
I16 = mybir.dt.int16


def build_program():
    nc = bacc.Bacc("TRN2", target_bir_lowering=False, debug=False,
                   num_devices=N_CORES)

    xt8_d = nc.dram_tensor("xt8", [E, S], FP8, kind="ExternalInput").ap()
    xqt8_d = nc.dram_tensor("xqt8", [E, QOWN], FP8, kind="ExternalInput").ap()
    xqtb_d = nc.dram_tensor("xqtb", [E, QOWN], BF16, kind="ExternalInput").ap()
    wq8_d = nc.dram_tensor("wq8", [E, E], FP8, kind="ExternalInput").ap()
    wk8_d = nc.dram_tensor("wk8", [E, E], FP8, kind="ExternalInput").ap()
    wv8_d = nc.dram_tensor("wv8", [E, E], FP8, kind="ExternalInput").ap()
    wvb_d = nc.dram_tensor("wvb", [E, E], BF16, kind="ExternalInput").ap()
    wob_d = nc.dram_tensor("wob", [D, H, E], BF16, kind="ExternalInput").ap()
    bob_d = nc.dram_tensor("bob", [P, E], F32, kind="ExternalInput").ap()
    btab_d = nc.dram_tensor("btab", [P, TOT_SLOTS], F32, kind="ExternalInput").ap()
    btabs_d = nc.dram_tensor("btabs", [P, TOT_SLOTS], F32, kind="ExternalInput").ap()
    dmask_d = nc.dram_tensor("dmask", [P, H * P], BF16, kind="ExternalInput").ap()
    y = nc.dram_tensor("y", [QOWN, E], F32, kind="ExternalOutput").ap()

    with tile.TileContext(nc) as tc, \
         tc.tile_pool(name="const", bufs=1) as const, \
         tc.tile_pool(name="big", bufs=1) as big, \
         tc.tile_pool(name="xstr", bufs=2) as xstr, \
         tc.tile_pool(name="scp", bufs=2, space="PSUM") as sc_p, \
         tc.tile_pool(name="ctxp", bufs=1, space="PSUM") as ctx_p, \
         tc.tile_pool(name="epip", bufs=1, space="PSUM") as epi_p, \
         tc.tile_pool(name="ptp", bufs=3) as pt_p, \
         tc.tile_pool(name="sintp", bufs=3) as sint_p, \
         tc.tile_pool(name="misc", bufs=1) as misc:

        # ---------------- constants / weights ----------------
        btab = const.tile([P, TOT_SLOTS], F32, tag="btab")
        nc.sync.dma_start(out=btab, in_=btab_d)
        btabs = const.tile([P, TOT_SLOTS], F32, tag="btabs")
        nc.sync.dma_start(out=btabs, in_=btabs_d)
        bob = const.tile([P, E], F32, tag="bob")
        nc.sync.dma_start(out=bob, in_=bob_d)
        dmask = const.tile([P, H * P], BF16, tag="dmask")
        nc.sync.dma_start(out=dmask, in_=dmask_d)

        wq8 = const.tile([P, 6, E], FP8, tag="wq8")
        nc.sync.dma_start(out=wq8, in_=wq8_d.rearrange("(c p) n -> p c n", p=P))
        wk8 = const.tile([P, 6, E], FP8, tag="wk8")
        nc.sync.dma_start(out=wk8, in_=wk8_d.rearrange("(c p) n -> p c n", p=P))
        wv8 = const.tile([P, 6, E], FP8, tag="wv8")
        nc.sync.dma_start(out=wv8, in_=wv8_d.rearrange("(c p) n -> p c n", p=P))
        wvb = const.tile([P, 6, E], BF16, tag="wvb")
        nc.sync.dma_start(out=wvb, in_=wvb_d.rearrange("(c p) n -> p c n", p=P))
        wob = const.tile([D, H, E], BF16, tag="wob")
        nc.sync.dma_start(out=wob, in_=wob_d)

        xqt8 = const.tile([P, 6, QOWN], FP8, tag="xqt8")
        nc.sync.dma_start(out=xqt8, in_=xqt8_d.rearrange("(c p) n -> p c n", p=P))
        xqtb = const.tile([P, 6, QOWN], BF16, tag="xqtb")
        nc.sync.dma_start(out=xqtb, in_=xqtb_d.rearrange("(c p) n -> p c n", p=P))

        zb = const.tile([P, 512], BF16, tag="zb")
        nc.vector.memset(zb, 0.0)

        # DR-group views (e-tiles paired: group g = e-chunks (2g, 2g+1))
        wq8v = wq8.rearrange("p (g t) n -> p g t n", g=3)
        wk8v = wk8.rearrange("p (g t) n -> p g t n", g=3)
        wv8v = wv8.rearrange("p (g t) n -> p g t n", g=3)
        xqt8v = xqt8.rearrange("p (g t) n -> p g t n", g=3)

        # ---------------- persistent operands ----------------
        kt8 = big.tile([P, 6, S + P], FP8, tag="kt8")   # K^T pairs, +pad col blk
        vt8 = big.tile([P, S // 256, 2, H, VST], FP8, tag="vt8")
        kto8 = big.tile([P, 6, QOWN + P], FP8, tag="kto8")
        vtob = big.tile([P, NCH, H, D + 1], BF16, tag="vtob")
        qtp8 = big.tile([P, 6, 2, NCH, 256], FP8, tag="qtp8")  # [pc, tile, chunk, var*q]
        ctxnb = misc.tile([D, H, P], BF16, tag="ctxnb", bufs=2)

        nc.gpsimd.memset(kt8[:, :, S:], 0.0)
        nc.gpsimd.memset(kto8[:, :, QOWN:], 0.0)
        nc.gpsimd.memset(qtp8[:, :, 1, :, :], 0.0)          # DR tile1 = 0
        nc.gpsimd.memset(qtp8[D:P, :, 0, :, 0:P], 0.0)      # var0: odd half = 0
        nc.gpsimd.memset(qtp8[0:D, :, 0, :, P:256], 0.0)    # var1: even half = 0
        nc.gpsimd.memset(vt8[:, :, :, :, D:], 0.0)
        nc.gpsimd.memset(vt8[:, :, :, :, D:D + 1], 64.0)    # 64*l ones col
        nc.gpsimd.memset(vtob[:, :, :, D:D + 1], 64.0)

        def cast(i, out, in_):
            # PSUM -> SBUF casts alternate DVE / ACT (GPSIMD can't read PSUM)
            if i % 2 == 0:
                nc.vector.tensor_copy(out=out, in_=in_)
            else:
                nc.scalar.copy(out=out, in_=in_)

        # ---------------- own projections (Q, diag K/V) ----------------
        nci = [0]

        def proj_own_q():
            for pc in range(6):
                ps = sc_p.tile([P, QOWN], F32, tag="sc", name=f"q_{pc}")
                for g in range(3):
                    nc.tensor.matmul(out=ps, lhsT=wq8v[:, g, :, pc * P:(pc + 1) * P],
                                     rhs=xqt8v[:, g], start=(g == 0), stop=(g == 2),
                                     perf_mode=DR)
                qv = qtp8.rearrange("p c t g (v q) -> p c t g v q", v=2)
                nc.scalar.copy(
                    out=qv[0:D, pc, 0, :, 0, :],
                    in_=ps[0:D, :].rearrange("p (g q) -> p g q", q=P))
                nc.scalar.copy(
                    out=qv[D:P, pc, 0, :, 1, :],
                    in_=ps[D:P, :].rearrange("p (g q) -> p g q", q=P))
        def proj_own_diag():
            for pc in range(6):
                ps = sc_p.tile([P, QOWN], F32, tag="sc", name=f"ko_{pc}")
                for g in range(3):
                    nc.tensor.matmul(out=ps, lhsT=wk8v[:, g, :, pc * P:(pc + 1) * P],
                                     rhs=xqt8v[:, g], start=(g == 0), stop=(g == 2),
                                     perf_mode=DR)
                cast(nci[0], kto8[:, pc, 0:QOWN], ps)
                nci[0] += 1
            # diag V, bf16 (wvb = 64*Wv so scale matches fp8 V path)
            for qb in range(NCH):
                ps = sc_p.tile([P, E], F32, tag="sc", name=f"vo_{qb}")
                for lo, n in ((0, 512), (512, 256)):
                    for ec in range(6):
                        nc.tensor.matmul(
                            out=ps[:, lo:lo + n],
                            lhsT=xqtb[:, ec, qb * P:(qb + 1) * P],
                            rhs=wvb[:, ec, lo:lo + n],
                            start=(ec == 0), stop=(ec == 5))
                cast(nci[0], vtob[:, qb, :, 0:D],
                     ps.rearrange("p (h d) -> p h d", d=D))
                nci[0] += 1

        # ---------------- replicated K/V projection (fp8-DR) ----------------
        proj_q = []

        def proj_kv_enqueue(sg):
            """Queue K^T/V projection tiles for x rows [1024*sg, ...)."""
            state = {}

            def load_x(sg=sg):
                xs = xstr.tile([P, 6, 1024], FP8, tag="xs", name=f"xs_{sg}")
                nc.sync.dma_start(
                    out=xs, in_=xt8_d.rearrange("(c p) s -> p c s", p=P)
                    [:, :, sg * 1024:(sg + 1) * 1024])
                state["xsv"] = xs.rearrange("p (g t) n -> p g t n", g=3)

            def k_tile(pc, sg=sg):
                xsv = state["xsv"]
                ps = sc_p.tile([P, 1024], F32, tag="sc", name=f"k_{sg}_{pc}")
                for half in range(2):
                    for g in range(3):
                        nc.tensor.matmul(
                            out=ps[:, half * 512:(half + 1) * 512],
                            lhsT=wk8v[:, g, :, pc * P:(pc + 1) * P],
                            rhs=xsv[:, g, :, half * 512:(half + 1) * 512],
                            start=(g == 0), stop=(g == 2), perf_mode=DR)
                cast(nci[0], kt8[:, pc, sg * 1024:(sg + 1) * 1024], ps)
                nci[0] += 1

            def v_tile(b, sg=sg):
                xsv = state["xsv"]
                sb = 8 * sg + b
                ps = sc_p.tile([P, E], F32, tag="sc", name=f"v_{sg}_{b}")
                for lo, n in ((0, 512), (512, 256)):
                    for g in range(3):
                        nc.tensor.matmul(
                            out=ps[:, lo:lo + n],
                            lhsT=xsv[:, g, :, b * P:(b + 1) * P],
                            rhs=wv8v[:, g, :, lo:lo + n],
                            start=(g == 0), stop=(g == 2), perf_mode=DR)
                cast(nci[0], vt8[:, sb // 2, sb % 2, :, 0:D],
                     ps.rearrange("p (h d) -> p h d", d=D))
                nci[0] += 1

            first = True
            for pc in range(6):
                if first:
                    proj_q.append(lambda pc=pc: (load_x(), k_tile(pc)))
                    first = False
                else:
                    proj_q.append(lambda pc=pc: k_tile(pc))
            for b in range(8):
                proj_q.append(lambda b=b: v_tile(b))

        def proj_drain(n):
            for _ in range(n):
                if proj_q:
                    proj_q.pop(0)()

        def proj_kv(sg):
            proj_kv_enqueue(sg)
            proj_drain(len(proj_q))

        # ---------------- attention ----------------
        def scores(g, s, lhs_tile, name):
            """fp8-DR scores for slot s of chunk g -> [128, 1536] psum pair."""
            scs = []
            for hg in range(2):
                sc = sc_p.tile([P, 768], F32, tag="sc", name=f"s{name}_{hg}")
                for pl in range(3):
                    pc = hg * 3 + pl
                    lhsT = lhs_tile[:, pc, s * P:s * P + 2 * P].rearrange(
                        "p (t k) -> p t k", t=2)
                    nc.tensor.matmul(out=sc[:, pl * 256:(pl + 1) * 256],
                                     lhsT=lhsT,
                                     rhs=qtp8[:, pc, :, g, :],
                                     start=True, stop=True, perf_mode=DR,
                                     skip_group_check=True)
                scs.append(sc)
            return scs

        exp_all_act = bool(int(os.environ.get("KV2_EXP_ALL_ACT", "0")))

        def exp_fp8(g, s, scs, dst, name):
            """exp -> fp8 into dst[:, 0:1536]: ACT heads 0-5, DVE+Pool 6-11."""
            si = SLOT_BASE[g] + s
            nc.scalar.activation(out=dst[:, 0:768], in_=scs[0], func=EXP,
                                 bias=btab[:, si:si + 1], scale=SC_SCALE)
            if exp_all_act:
                nc.scalar.activation(out=dst[:, 768:1536], in_=scs[1], func=EXP,
                                     bias=btab[:, si:si + 1], scale=SC_SCALE)
                return
            sint = sint_p.tile([P, 768], I16, tag="sint", name=f"si{name}")
            nc.vector.tensor_scalar(out=sint, in0=scs[1],
                                    scalar1=float(SCHR_A),
                                    scalar2=btabs[:, si:si + 1],
                                    op0=mybir.AluOpType.mult,
                                    op1=mybir.AluOpType.add)
            nc.gpsimd.tensor_copy(out=dst[:, 768:1408],
                                  in_=sint.bitcast(BF16)[:, 0:640])
            nc.vector.tensor_copy(out=dst[:, 1408:1536],
                                  in_=sint.bitcast(BF16)[:, 640:768])

        def attn_chunk(g, prev_epi=None):
            nslot = SLOTS[g]
            npair = (nslot - 2) // 2
            ctxt = [ctx_p.tile([D + 1, 4 * P], F32, tag=f"ctx{i}",
                                name=f"ctx{g}_{i}") for i in range(3)]

            def cx(h):
                return ctxt[h // 4][:, (h % 4) * P:(h % 4 + 1) * P]

            def ctx_fence(start):
                # one start/stop group per PSUM bank; real attnV matmuls are
                # flags=0 accumulates in between (interleaved OPEN groups in a
                # bank corrupt each other on HW)
                for i in range(3):
                    nc.tensor.matmul(out=ctxt[i], lhsT=zb[0:1, 0:D + 1],
                                     rhs=zb[0:1, 0:4 * P], start=start,
                                     stop=not start, skip_group_check=True)

            def attnv_pair(j):
                pt = pts[j % 3]
                for h in range(H):
                    nc.tensor.matmul(
                        out=cx(h), lhsT=vt8[:, j, :, h, 0:D + 1],
                        rhs=pt[:, :, h * P:(h + 1) * P],
                        start=False, stop=False, perf_mode=DR,
                        skip_group_check=True)

            ctx_fence(start=True)
            pts = {}
            for j in range(npair):
                pt = pt_p.tile([P, 2, H * P], FP8, tag="pt", name=f"pt{g}_{j}")
                pts[j % 3] = pt
                scs = scores(g, 2 * j, kt8, f"{g}_{2 * j}")
                exp_fp8(g, 2 * j, scs, pt[:, 0, :], f"{g}_{2 * j}")
                if j == 1 and prev_epi is not None:
                    prev_epi()  # previous chunk's epilogue rides this chunk's pipeline
                if j > 0:
                    attnv_pair(j - 1)
                scs = scores(g, 2 * j + 1, kt8, f"{g}_{2 * j + 1}")
                exp_fp8(g, 2 * j + 1, scs, pt[:, 1, :], f"{g}_{2 * j + 1}")
            attnv_pair(npair - 1)

            # unpaired regular slot (plain fp8 matmuls)
            s = nslot - 2
            scs = scores(g, s, kt8, f"{g}_u")
            ptu = pt_p.tile([P, 2, H * P], FP8, tag="pt", name=f"ptu{g}")
            exp_fp8(g, s, scs, ptu[:, 0, :], f"{g}_u")
            for h in range(H):
                nc.tensor.matmul(out=cx(h),
                                 lhsT=vt8[:, s // 2, s % 2, h, 0:D + 1],
                                 rhs=ptu[:, 0, h * P:(h + 1) * P],
                                 start=False, stop=False,
                                 skip_group_check=True)

            # diagonal slot: fp8 scores from own K, bf16 P & V
            scs = scores(g, g, kto8, f"{g}_d")
            ptd = misc.tile([P, H * P], BF16, tag="ptd", bufs=2, name=f"ptd{g}")
            si = SLOT_BASE[g] + nslot - 1
            for hg in range(2):
                nc.scalar.activation(out=ptd[:, hg * 768:(hg + 1) * 768],
                                     in_=scs[hg], func=EXP,
                                     bias=btab[:, si:si + 1], scale=SC_SCALE)
            ptm = misc.tile([P, H * P], BF16, tag="ptm", bufs=2, name=f"ptm{g}")
            nc.vector.tensor_mul(out=ptm, in0=ptd, in1=dmask)
            for h in range(H):
                nc.tensor.matmul(out=cx(h), lhsT=vtob[:, g, h, :],
                                 rhs=ptm[:, h * P:(h + 1) * P],
                                 start=False, stop=False,
                                 skip_group_check=True)
            ctx_fence(start=False)

            # ---------------- epilogue (deferred: emitted inside next chunk) --
            def epilogue():
                lst = misc.tile([P, H * P], F32, tag="lst", name=f"lst{g}")
                for i in range(3):
                    nc.scalar.copy(
                        out=lst[D:D + 1, i * 4 * P:(i + 1) * 4 * P],
                        in_=ctxt[i][D:D + 1, :])
                nc.sync.dma_start(out=lst[0:1, :], in_=lst[D:D + 1, :])
                linv = misc.tile([P, H * P], F32, tag="linv", name=f"linv{g}")
                nc.vector.reciprocal_approx_fast(out=linv[0:1, :], in_=lst[0:1, :])
                bcs = misc.tile([D, H * P], F32, tag="bcs", bufs=2,
                                name=f"bcs{g}")
                nc.gpsimd.partition_broadcast(bcs, linv[0:1, :])
                for h in range(H):
                    nc.vector.tensor_mul(
                        out=ctxnb[:, h, :], in0=cx(h)[0:D, :],
                        in1=bcs[:, h * P:(h + 1) * P])
                for fh in range(2):
                    yp = epi_p.tile([P, 384], F32, tag="epi", name=f"y{g}_{fh}")
                    for h in range(H):
                        nc.tensor.matmul(out=yp, lhsT=ctxnb[:, h, :],
                                         rhs=wob[:, h, fh * 384:(fh + 1) * 384],
                                         start=(h == 0), stop=(h == H - 1))
                    outs = misc.tile([P, 384], F32, tag="outs", bufs=2,
                                     name=f"o{g}_{fh}")
                    nc.vector.tensor_add(out=outs, in0=yp,
                                         in1=bob[:, fh * 384:(fh + 1) * 384])
                    nc.sync.dma_start(
                        out=y[g * P:(g + 1) * P, fh * 384:(fh + 1) * 384],
                        in_=outs)
            return epilogue

        # ---------------- emission: interleave proj and attention ----------
        proj_own_q()
        proj_kv(0)
        proj_own_diag()
        epi = attn_chunk(0)
        proj_kv(1)
        epi = attn_chunk(1, epi)
        proj_kv(2)
        epi = attn_chunk(2, epi)
        proj_kv(3)
        epi = attn_chunk(3, epi)
        epi()

    nc.compile()
    return nc


_NC_CACHE = None


def _get_program():
    global _NC_CACHE
    if _NC_CACHE is None:
        _NC_CACHE = build_program()
    return _NC_CACHE


def _host_inputs(x, Wq, Wk, Wv, Wo, bo):
    x = np.ascontiguousarray(x.reshape(S, E), dtype=np.float32)
    f8 = ml_dtypes.float8_e4m3
    bf = ml_dtypes.bfloat16

    xT = np.ascontiguousarray(x.T)                      # [E, S]
    xt8 = xT.astype(f8)
    wq8 = np.ascontiguousarray(Wq * 64).astype(f8)
    wk8 = np.ascontiguousarray(Wk * 64).astype(f8)
    wv8 = np.ascontiguousarray(Wv * 64).astype(f8)
    wvb = np.ascontiguousarray(Wv * 64).astype(bf)
    wob = np.ascontiguousarray(
        Wo.reshape(H, D, E).transpose(1, 0, 2)).astype(bf)
    bob = np.ascontiguousarray(np.broadcast_to(bo.astype(np.float32), (P, E)))
    tri = (np.arange(P)[:, None] <= np.arange(P)[None, :]).astype(np.float32)
    dmask = np.ascontiguousarray(np.tile(tri, (1, H)).astype(bf))

    in_maps = []
    for c in range(N_CORES):
        chunks = [8 * g + c for g in range(NCH)]
        cols = np.concatenate([np.arange(gc * P, (gc + 1) * P) for gc in chunks])
        xqT = np.ascontiguousarray(xT[:, cols])
        btab = np.zeros((P, TOT_SLOTS), dtype=np.float32)
        btabs = np.zeros((P, TOT_SLOTS), dtype=np.float32)
        for g in range(NCH):
            diagk = chunks[g]
            for s in range(SLOTS[g]):
                v = 0.0 if (s == SLOTS[g] - 1 or s < diagk) else NEG
                btab[:, SLOT_BASE[g] + s] = v
                btabs[:, SLOT_BASE[g] + s] = SCHR_B0 + \
                    (2.0 ** 7) / np.log(2.0) * v
        in_maps.append({
            "xt8": xt8,
            "xqt8": xqT.astype(f8),
            "xqtb": xqT.astype(bf),
            "wq8": wq8, "wk8": wk8, "wv8": wv8, "wvb": wvb,
            "wob": wob, "bob": bob,
            "btab": btab, "btabs": btabs, "dmask": dmask,
        })
    return in_maps


def kernel(x, Wq, Wk, Wv, Wo, bo, mask=None, **_ignored):
    nc = _get_program()
    in_maps = _host_inputs(
        np.asarray(x), np.asarray(Wq), np.asarray(Wk), np.asarray(Wv),
        np.asarray(Wo), np.asarray(bo),
    )
    trace = bool(int(os.environ.get("BASS_KERNEL_TRACE", "0")))
    res = bass_utils.run_bass_kernel_spmd(
        nc, in_maps, core_ids=list(range(N_CORES)), trace=trace
    )
    if trace:
        kernel.last_results = res
    out = np.empty((S, E), dtype=np.float32)
    for c in range(N_CORES):
        yc = res.results[c]["y"]
        for g in range(NCH):
            gc = 8 * g + c
            out[gc * P:(gc + 1) * P] = yc[g * P:(g + 1) * P]
    return out.reshape(1, S, E)


# revision 4
# speedup vs baseline: 296.7494x; 1.0206x over previous
"""Trainium2 Bass kernel v2 for nn_MultiHeadAttention (B=1, S=4096, E=768, H=12, D=64).

Causal MHA, sequence-parallel across 8 cores (chunk g of core c = global
q-chunk 8g+c), fp8-heavy compute:

- Q/K projections + scores in fp8e4 with DoubleRow perf mode (4x bf16
  throughput; weights pre-scaled x64 on host so fp8 dynamic range is used).
- Off-diagonal attn@V in fp8-DR over slot pairs (V from fp8 projection,
  ones-column carries the softmax denominator l).
- Diagonal (and one unpaired) slot in higher precision: P bf16, V bf16 from
  a bf16 projection - kills the early-row error spike where softmax is
  peaked and ctx inherits a single v's quantization error.
- exp split across three engines per slot: ACT does heads 0-5 natively
  (fp8 out); DVE computes heads 6-11 via the Schraudolph bit trick
  (i32 = sc*A + B, bits == fp32 exp) and Pool casts those to fp8.
- Causally dead slots killed by exp bias -60 (data-driven, uniform program).
- Out-projection bf16, softmax 1/l via Ln/Exp + f32r ones-matmul broadcast.
"""

import os

import numpy as np
import ml_dtypes

import concourse.bass as bass
import concourse.tile as tile
from concourse import bacc, bass_utils, mybir

F32 = mybir.dt.float32
F32R = mybir.dt.float32r
BF16 = mybir.dt.bfloat16
FP8 = mybir.dt.float8e4
DR = mybir.MatmulPerfMode.DoubleRow
EXP = mybir.ActivationFunctionType.Exp

N_CORES = 8
S, E, H, D = 4096, 768, 12, 64
P = 128
NCH = 4
SLOTS = [8, 16, 24, 32]
SLOT_BASE = [0, 8, 24, 48]
TOT_SLOTS = 80
QOWN = NCH * P
NEG = -60.0
VST = 68  # vt8 head stride: DR needs tile step (12*VST) 16B-aligned

SC_SCALE = 0.125 / (64.0 * 64.0)  # scores psum = 4096 * (q.k); exp scale
# int16 Schraudolph: i16 = sc*A16 + B16; i16 bits are the bf16 of exp(sc*scale)
SCHR_A = (2.0 ** 7) / np.log(2.0) * SC_SCALE
SCHR_C = 366392.0 / 65536.0
SCHR_B0 = 127.0 * 2.0 ** 7 - SCHR_C
LN64 = float(np.log(64.0))
I16 = mybir.dt.int16


def build_program():
    nc = bacc.Bacc("TRN2", target_bir_lowering=False, debug=False,
                   num_devices=N_CORES)

    xt8_d = nc.dram_tensor("xt8", [E, S], FP8, kind="ExternalInput").ap()
    xqt8_d = nc.dram_tensor("xqt8", [E, QOWN], FP8, kind="ExternalInput").ap()
    xqtb_d = nc.dram_tensor("xqtb", [E, QOWN], BF16, kind="ExternalInput").ap()
    wq8_d = nc.dram_tensor("wq8", [E, E], FP8, kind="ExternalInput").ap()
    wk8_d = nc.dram_tensor("wk8", [E, E], FP8, kind="ExternalInput").ap()
    wv8_d = nc.dram_tensor("wv8", [E, E], FP8, kind="ExternalInput").ap()
    wvb_d = nc.dram_tensor("wvb", [E, E], BF16, kind="ExternalInput").ap()
    wob_d = nc.dram_tensor("wob", [D, H, E], BF16, kind="ExternalInput").ap()
    bob_d = nc.dram_tensor("bob", [P, E], F32, kind="ExternalInput").ap()
    btab_d = nc.dram_tensor("btab", [P, TOT_SLOTS], F32, kind="ExternalInput").ap()
    btabs_d = nc.dram_tensor("btabs", [P, TOT_SLOTS], F32, kind="ExternalInput").ap()
    dmask_d = nc.dram_tensor("dmask", [P, H * P], BF16, kind="ExternalInput").ap()
    y = nc.dram_tensor("y", [QOWN, E], F32, kind="ExternalOutput").ap()

    with tile.TileContext(nc) as tc, \
         tc.tile_pool(name="const", bufs=1) as const, \
         tc.tile_pool(name="big", bufs=1) as big, \
         tc.tile_pool(name="xstr", bufs=2) as xstr, \
         tc.tile_pool(name="scp", bufs=2, space="PSUM") as sc_p, \
         tc.tile_pool(name="ctxp", bufs=1, space="PSUM") as ctx_p, \
         tc.tile_pool(name="epip", bufs=1, space="PSUM") as epi_p, \
         tc.tile_pool(name="ptp", bufs=3) as pt_p, \
         tc.tile_pool(name="sintp", bufs=3) as sint_p, \
         tc.tile_pool(name="misc", bufs=1) as misc:

        # ------- loads ordered by first use: Q/K/V proj inputs first -------
        xqt8 = const.tile([P, 6, QOWN], FP8, tag="xqt8")
        nc.sync.dma_start(out=xqt8, in_=xqt8_d.rearrange("(c p) n -> p c n", p=P))
        wq8 = const.tile([P, 6, E], FP8, tag="wq8")
        nc.sync.dma_start(out=wq8, in_=wq8_d.rearrange("(c p) n -> p c n", p=P))
        wk8 = const.tile([P, 6, E], FP8, tag="wk8")
        nc.sync.dma_start(out=wk8, in_=wk8_d.rearrange("(c p) n -> p c n", p=P))
        wv8 = const.tile([P, 6, E], FP8, tag="wv8")
        nc.sync.dma_start(out=wv8, in_=wv8_d.rearrange("(c p) n -> p c n", p=P))
        btab = const.tile([P, TOT_SLOTS], F32, tag="btab")
        nc.sync.dma_start(out=btab, in_=btab_d)
        btabs = const.tile([P, TOT_SLOTS], F32, tag="btabs")
        nc.sync.dma_start(out=btabs, in_=btabs_d)
        # late-needed: diag V proj, diag mask, out-proj weights, bias
        xqtb = const.tile([P, 6, QOWN], BF16, tag="xqtb")
        nc.sync.dma_start(out=xqtb, in_=xqtb_d.rearrange("(c p) n -> p c n", p=P))
        wvb = const.tile([P, 6, E], BF16, tag="wvb")
        nc.sync.dma_start(out=wvb, in_=wvb_d.rearrange("(c p) n -> p c n", p=P))
        dmask = const.tile([P, H * P], BF16, tag="dmask")
        nc.sync.dma_start(out=dmask, in_=dmask_d)
        wob = const.tile([D, H, E], BF16, tag="wob")
        nc.sync.dma_start(out=wob, in_=wob_d)
        bob = const.tile([P, E], F32, tag="bob")
        nc.sync.dma_start(out=bob, in_=bob_d)

        zb = const.tile([P, 512], BF16, tag="zb")
        nc.vector.memset(zb, 0.0)

        # DR-group views (e-tiles paired: group g = e-chunks (2g, 2g+1))
        wq8v = wq8.rearrange("p (g t) n -> p g t n", g=3)
        wk8v = wk8.rearrange("p (g t) n -> p g t n", g=3)
        wv8v = wv8.rearrange("p (g t) n -> p g t n", g=3)
        xqt8v = xqt8.rearrange("p (g t) n -> p g t n", g=3)

        # ---------------- persistent operands ----------------
        kt8 = big.tile([P, 6, S + P], FP8, tag="kt8")   # K^T pairs, +pad col blk
        vt8 = big.tile([P, S // 256, 2, H, VST], FP8, tag="vt8")
        kto8 = big.tile([P, 6, QOWN + P], FP8, tag="kto8")
        vtob = big.tile([P, NCH, H, D + 1], BF16, tag="vtob")
        qtp8 = big.tile([P, 6, 2, NCH, 256], FP8, tag="qtp8")  # [pc, tile, chunk, var*q]
        ctxnb = misc.tile([D, H, P], BF16, tag="ctxnb", bufs=2)

        nc.gpsimd.memset(kt8[:, :, S:], 0.0)
        nc.gpsimd.memset(kto8[:, :, QOWN:], 0.0)
        nc.gpsimd.memset(qtp8[:, :, 1, :, :], 0.0)          # DR tile1 = 0
        nc.gpsimd.memset(qtp8[D:P, :, 0, :, 0:P], 0.0)      # var0: odd half = 0
        nc.gpsimd.memset(qtp8[0:D, :, 0, :, P:256], 0.0)    # var1: even half = 0
        nc.gpsimd.memset(vt8[:, :, :, :, D:], 0.0)
        nc.gpsimd.memset(vt8[:, :, :, :, D:D + 1], 64.0)    # 64*l ones col
        nc.gpsimd.memset(vtob[:, :, :, D:D + 1], 64.0)

        def cast(i, out, in_):
            # PSUM -> SBUF casts alternate DVE / ACT (GPSIMD can't read PSUM)
            if i % 2 == 0:
                nc.vector.tensor_copy(out=out, in_=in_)
            else:
                nc.scalar.copy(out=out, in_=in_)

        # ---------------- own projections (Q, diag K/V) ----------------
        nci = [0]

        def proj_own_q():
            for pc in range(6):
                ps = sc_p.tile([P, QOWN], F32, tag="sc", name=f"q_{pc}")
                for g in range(3):
                    nc.tensor.matmul(out=ps, lhsT=wq8v[:, g, :, pc * P:(pc + 1) * P],
                                     rhs=xqt8v[:, g], start=(g == 0), stop=(g == 2),
                                     perf_mode=DR)
                qv = qtp8.rearrange("p c t g (v q) -> p c t g v q", v=2)
                nc.scalar.copy(
                    out=qv[0:D, pc, 0, :, 0, :],
                    in_=ps[0:D, :].rearrange("p (g q) -> p g q", q=P))
                nc.scalar.copy(
                    out=qv[D:P, pc, 0, :, 1, :],
                    in_=ps[D:P, :].rearrange("p (g q) -> p g q", q=P))
        def proj_own_diag():
            for pc in range(6):
                ps = sc_p.tile([P, QOWN], F32, tag="sc", name=f"ko_{pc}")
                for g in range(3):
                    nc.tensor.matmul(out=ps, lhsT=wk8v[:, g, :, pc * P:(pc + 1) * P],
                                     rhs=xqt8v[:, g], start=(g == 0), stop=(g == 2),
                                     perf_mode=DR)
                cast(nci[0], kto8[:, pc, 0:QOWN], ps)
                nci[0] += 1
            # diag V, bf16 (wvb = 64*Wv so scale matches fp8 V path)
            for qb in range(NCH):
                ps = sc_p.tile([P, E], F32, tag="sc", name=f"vo_{qb}")
                for lo, n in ((0, 512), (512, 256)):
                    for ec in range(6):
                        nc.tensor.matmul(
                            out=ps[:, lo:lo + n],
                            lhsT=xqtb[:, ec, qb * P:(qb + 1) * P],
                            rhs=wvb[:, ec, lo:lo + n],
                            start=(ec == 0), stop=(ec == 5))
                cast(nci[0], vtob[:, qb, :, 0:D],
                     ps.rearrange("p (h d) -> p h d", d=D))
                nci[0] += 1

        # ---------------- replicated K/V projection (fp8-DR) ----------------
        proj_q = []

        def proj_kv_enqueue(sg):
            """Queue K^T/V projection tiles for x rows [1024*sg, ...)."""
            state = {}

            def load_x(sg=sg):
                xs = xstr.tile([P, 6, 1024], FP8, tag="xs", name=f"xs_{sg}")
                nc.sync.dma_start(
                    out=xs, in_=xt8_d.rearrange("(c p) s -> p c s", p=P)
                    [:, :, sg * 1024:(sg + 1) * 1024])
                state["xsv"] = xs.rearrange("p (g t) n -> p g t n", g=3)

            def k_tile(pc, sg=sg):
                xsv = state["xsv"]
                ps = sc_p.tile([P, 1024], F32, tag="sc", name=f"k_{sg}_{pc}")
                for half in range(2):
                    for g in range(3):
                        nc.tensor.matmul(
                            out=ps[:, half * 512:(half + 1) * 512],
                            lhsT=wk8v[:, g, :, pc * P:(pc + 1) * P],
                            rhs=xsv[:, g, :, half * 512:(half + 1) * 512],
                            start=(g == 0), stop=(g == 2), perf_mode=DR)
                cast(nci[0], kt8[:, pc, sg * 1024:(sg + 1) * 1024], ps)
                nci[0] += 1

            def v_tile(b, sg=sg):
                xsv = state["xsv"]
                sb = 8 * sg + b
                ps = sc_p.tile([P, E], F32, tag="sc", name=f"v_{sg}_{b}")
                for lo, n in ((0, 512), (512, 256)):
                    for g in range(3):
                        nc.tensor.matmul(
                            out=ps[:, lo:lo + n],
                            lhsT=xsv[:, g, :, b * P:(b + 1) * P],
                            rhs=wv8v[:, g, :, lo:lo + n],
                            start=(g == 0), stop=(g == 2), perf_mode=DR)
                cast(nci[0], vt8[:, sb // 2, sb % 2, :, 0:D],
                     ps.rearrange("p (h d) -> p h d", d=D))
                nci[0] += 1

            first = True
            for pc in range(6):
                if first:
                    proj_q.append(lambda pc=pc: (load_x(), k_tile(pc)))
                    first = False
                else:
                    proj_q.append(lambda pc=pc: k_tile(pc))
            for b in range(8):
                proj_q.append(lambda b=b: v_tile(b))

        def proj_drain(n):
            for _ in range(n):
                if proj_q:
                    proj_q.pop(0)()

        def proj_kv(sg):
            proj_kv_enqueue(sg)
            proj_drain(len(proj_q))

        # ---------------- attention ----------------
        def scores(g, s, lhs_tile, name):
            """fp8-DR scores for slot s of chunk g -> [128, 1536] psum pair."""
            scs = []
            for hg in range(2):
                sc = sc_p.tile([P, 768], F32, tag="sc", name=f"s{name}_{hg}")
                for pl in range(3):
                    pc = hg * 3 + pl
                    lhsT = lhs_tile[:, pc, s * P:s * P + 2 * P].rearrange(
                        "p (t k) -> p t k", t=2)
                    nc.tensor.matmul(out=sc[:, pl * 256:(pl + 1) * 256],
                                     lhsT=lhsT,
                                     rhs=qtp8[:, pc, :, g, :],
                                     start=True, stop=True, perf_mode=DR,
                                     skip_group_check=True)
                scs.append(sc)
            return scs

        exp_all_act = bool(int(os.environ.get("KV2_EXP_ALL_ACT", "0")))

        def exp_fp8(g, s, scs, dst, name):
            """exp -> fp8 into dst[:, 0:1536]: ACT heads 0-5, DVE+Pool 6-11."""
            si = SLOT_BASE[g] + s
            nc.scalar.activation(out=dst[:, 0:768], in_=scs[0], func=EXP,
                                 bias=btab[:, si:si + 1], scale=SC_SCALE)
            if exp_all_act:
                nc.scalar.activation(out=dst[:, 768:1536], in_=scs[1], func=EXP,
                                     bias=btab[:, si:si + 1], scale=SC_SCALE)
                return
            sint = sint_p.tile([P, 768], I16, tag="sint", name=f"si{name}")
            nc.vector.tensor_scalar(out=sint, in0=scs[1],
                                    scalar1=float(SCHR_A),
                                    scalar2=btabs[:, si:si + 1],
                                    op0=mybir.AluOpType.mult,
                                    op1=mybir.AluOpType.add)
            nc.gpsimd.tensor_copy(out=dst[:, 768:1536], in_=sint.bitcast(BF16))

        def attn_chunk(g, prev_epi=None):
            nslot = SLOTS[g]
            npair = (nslot - 2) // 2
            ctxt = [ctx_p.tile([D + 1, 4 * P], F32, tag=f"ctx{i}",
                                name=f"ctx{g}_{i}") for i in range(3)]

            def cx(h):
                return ctxt[h // 4][:, (h % 4) * P:(h % 4 + 1) * P]

            def ctx_fence(start):
                # one start/stop group per PSUM bank; real attnV matmuls are
                # flags=0 accumulates in between (interleaved OPEN groups in a
                # bank corrupt each other on HW)
                for i in range(3):
                    nc.tensor.matmul(out=ctxt[i], lhsT=zb[0:1, 0:D + 1],
                                     rhs=zb[0:1, 0:4 * P], start=start,
                                     stop=not start, skip_group_check=True)

            def attnv_pair(j):
                pt = pts[j % 3]
                for h in range(H):
                    nc.tensor.matmul(
                        out=cx(h), lhsT=vt8[:, j, :, h, 0:D + 1],
                        rhs=pt[:, :, h * P:(h + 1) * P],
                        start=False, stop=False, perf_mode=DR,
                        skip_group_check=True)

            ctx_fence(start=True)
            pts = {}
            for j in range(npair):
                pt = pt_p.tile([P, 2, H * P], FP8, tag="pt", name=f"pt{g}_{j}")
                pts[j % 3] = pt
                scs = scores(g, 2 * j, kt8, f"{g}_{2 * j}")
                exp_fp8(g, 2 * j, scs, pt[:, 0, :], f"{g}_{2 * j}")
                if j == 1 and prev_epi is not None:
                    prev_epi()  # previous chunk's epilogue rides this chunk's pipeline
                if j > 0:
                    attnv_pair(j - 1)
                scs = scores(g, 2 * j + 1, kt8, f"{g}_{2 * j + 1}")
                exp_fp8(g, 2 * j + 1, scs, pt[:, 1, :], f"{g}_{2 * j + 1}")
            attnv_pair(npair - 1)

            # unpaired regular slot (plain fp8 matmuls)
            s = nslot - 2
            scs = scores(g, s, kt8, f"{g}_u")
            ptu = pt_p.tile([P, 2, H * P], FP8, tag="pt", name=f"ptu{g}")
            exp_fp8(g, s, scs, ptu[:, 0, :], f"{g}_u")
            for h in range(H):
                nc.tensor.matmul(out=cx(h),
                                 lhsT=vt8[:, s // 2, s % 2, h, 0:D + 1],
                                 rhs=ptu[:, 0, h * P:(h + 1) * P],
                                 start=False, stop=False,
                                 skip_group_check=True)

            # diagonal slot: fp8 scores from own K, bf16 P & V
            scs = scores(g, g, kto8, f"{g}_d")
            ptd = misc.tile([P, H * P], BF16, tag="ptd", bufs=2, name=f"ptd{g}")
            si = SLOT_BASE[g] + nslot - 1
            for hg in range(2):
                nc.scalar.activation(out=ptd[:, hg * 768:(hg + 1) * 768],
                                     in_=scs[hg], func=EXP,
                                     bias=btab[:, si:si + 1], scale=SC_SCALE)
            ptm = misc.tile([P, H * P], BF16, tag="ptm", bufs=2, name=f"ptm{g}")
            nc.vector.tensor_mul(out=ptm, in0=ptd, in1=dmask)
            for h in range(H):
                nc.tensor.matmul(out=cx(h), lhsT=vtob[:, g, h, :],
                                 rhs=ptm[:, h * P:(h + 1) * P],
                                 start=False, stop=False,
                                 skip_group_check=True)
            ctx_fence(start=False)

            # ---------------- epilogue (deferred: emitted inside next chunk) --
            def epilogue():
                lst = misc.tile([P, H * P], F32, tag="lst", name=f"lst{g}")
                for i in range(3):
                    nc.scalar.copy(
                        out=lst[D:D + 1, i * 4 * P:(i + 1) * 4 * P],
                        in_=ctxt[i][D:D + 1, :])
                nc.sync.dma_start(out=lst[0:1, :], in_=lst[D:D + 1, :])
                linv = misc.tile([P, H * P], F32, tag="linv", name=f"linv{g}")
                nc.vector.reciprocal_approx_fast(out=linv[0:1, :], in_=lst[0:1, :])
                bcs = misc.tile([D, H * P], F32, tag="bcs", bufs=2,
                                name=f"bcs{g}")
                nc.gpsimd.partition_broadcast(bcs, linv[0:1, :])
                for h in range(H):
                    nc.vector.tensor_mul(
                        out=ctxnb[:, h, :], in0=cx(h)[0:D, :],
                        in1=bcs[:, h * P:(h + 1) * P])
                for fh in range(2):
                    yp = epi_p.tile([P, 384], F32, tag="epi", name=f"y{g}_{fh}")
                    for h in range(H):
                        nc.tensor.matmul(out=yp, lhsT=ctxnb[:, h, :],
                                         rhs=wob[:, h, fh * 384:(fh + 1) * 384],
                                         start=(h == 0), stop=(h == H - 1))
                    outs = misc.tile([P, 384], F32, tag="outs", bufs=2,
                                     name=f"o{g}_{fh}")
                    nc.vector.tensor_add(out=outs, in0=yp,
                                         in1=bob[:, fh * 384:(fh + 1) * 384])
                    nc.sync.dma_start(
                        out=y[g * P:(g + 1) * P, fh * 384:(fh + 1) * 384],
                        in_=outs)
            return epilogue

        # ---------------- emission: interleave proj and attention ----------
        proj_own_q()
        proj_kv(0)
        proj_own_diag()
        epi = attn_chunk(0)
        proj_kv(1)
        epi = attn_chunk(1, epi)
        proj_kv(2)
        epi = attn_chunk(2, epi)
        proj_kv(3)
        epi = attn_chunk(3, epi)
        epi()

    nc.compile()
    return nc


_NC_CACHE = None


def _get_program():
    global _NC_CACHE
    if _NC_CACHE is None:
        _NC_CACHE = build_program()
    return _NC_CACHE


def _host_inputs(x, Wq, Wk, Wv, Wo, bo):
    x = np.ascontiguousarray(x.reshape(S, E), dtype=np.float32)
    f8 = ml_dtypes.float8_e4m3
    bf = ml_dtypes.bfloat16

    xT = np.ascontiguousarray(x.T)                      # [E, S]
    xt8 = xT.astype(f8)
    wq8 = np.ascontiguousarray(Wq * 64).astype(f8)
    wk8 = np.ascontiguousarray(Wk * 64).astype(f8)
    wv8 = np.ascontiguousarray(Wv * 64).astype(f8)
    wvb = np.ascontiguousarray(Wv * 64).astype(bf)
    wob = np.ascontiguousarray(
        Wo.reshape(H, D, E).transpose(1, 0, 2)).astype(bf)
    bob = np.ascontiguousarray(np.broadcast_to(bo.astype(np.float32), (P, E)))
    tri = (np.arange(P)[:, None] <= np.arange(P)[None, :]).astype(np.float32)
    dmask = np.ascontiguousarray(np.tile(tri, (1, H)).astype(bf))

    in_maps = []
    for c in range(N_CORES):
        chunks = [8 * g + c for g in range(NCH)]
        cols = np.concatenate([np.arange(gc * P, (gc + 1) * P) for gc in chunks])
        xqT = np.ascontiguousarray(xT[:, cols])
        btab = np.zeros((P, TOT_SLOTS), dtype=np.float32)
        btabs = np.zeros((P, TOT_SLOTS), dtype=np.float32)
        for g in range(NCH):
            diagk = chunks[g]
            for s in range(SLOTS[g]):
                v = 0.0 if (s == SLOTS[g] - 1 or s < diagk) else NEG
                btab[:, SLOT_BASE[g] + s] = v
                btabs[:, SLOT_BASE[g] + s] = SCHR_B0 + \
                    (2.0 ** 7) / np.log(2.0) * v
        in_maps.append({
            "xt8": xt8,
            "xqt8": xqT.astype(f8),
            "xqtb": xqT.astype(bf),
            "wq8": wq8, "wk8": wk8, "wv8": wv8, "wvb": wvb,
            "wob": wob, "bob": bob,
            "btab": btab, "btabs": btabs, "dmask": dmask,
        })
    return in_maps


def kernel(x, Wq, Wk, Wv, Wo, bo, mask=None, **_ignored):
    nc = _get_program()
    in_maps = _host_inputs(
        np.asarray(x), np.asarray(Wq), np.asarray(Wk), np.asarray(Wv),
        np.asarray(Wo), np.asarray(bo),
    )
    trace = bool(int(os.environ.get("BASS_KERNEL_TRACE", "0")))
    res = bass_utils.run_bass_kernel_spmd(
        nc, in_maps, core_ids=list(range(N_CORES)), trace=trace
    )
    if trace:
        kernel.last_results = res
    out = np.empty((S, E), dtype=np.float32)
    for c in range(N_CORES):
        yc = res.results[c]["y"]
        for g in range(NCH):
            gc = 8 * g + c
            out[gc * P:(gc + 1) * P] = yc[g * P:(g + 1) * P]
    return out.reshape(1, S, E)
